# revision 1
# baseline (speedup 1.0000x reference)
"""Trainium2 Bass kernel for LoopyBeliefPropagation (3-iter, mask=ones).

Math: for each (b, h) slice define tile[d,s] = s_sib[b,d,h,s],
SP = softplus(tile) - ln2, F[d,s] = (s!=h)(s!=d), Pdiff[d] =
s_edge[b,d,h,1] - s_edge[b,d,h,0].  Tracking only the message channel
difference delta = m_sib[...,1] - m_sib[...,0] collapses the reference's
3-iteration loop into closed form:

  r0 = Pdiff
  r1 = Pdiff + r0*NF + CPF
  r2 = Pdiff + r1*NF - SF(r0) - SPF + CPF
  bdiff = Pdiff + (r2+r0)*NF - SF(r1) + 2*CPF - SPF
  out[b,d,h,1] = sigmoid(bdiff), out[b,d,h,0] = sigmoid(-bdiff)

with NF[d] = sum_s F, SPF[d] = sum_s SP[d,s]F[d,s],
CPF[d] = sum_s SP[s,d]F[d,s], SF(v)[d] = sum_s v[s]F[d,s].
SPF decomposes into row sums (VectorE reduce) minus the h-column and
diagonal; CPF into column sums (one TensorE matmul per slice against a
ones column) minus row h and the diagonal.  The h-column / diagonal /
row-h values are tiny host-gathered side inputs.  SF(v) needs only a
per-slice scalar broadcast (matmul with an all-ones stationary).
softplus = Ln(Exp(x) + 1) using the natural_log_exp ACT table (this
toolchain has no softplus PWP table); the +1 rides the Ln bias.
No [h,d,s,B,2] intermediate is ever materialized.

Sharding: 8 cores x (b in 0..3, h-half in {0:64, 64:128}).  Each core
streams its 4 MiB s_sib shard once.
"""

import numpy as np

L = 128
H = 64            # h-slices per core
CH = 16           # h-slices per streamed chunk
NCHUNK = H // CH
N_CORES = 8
LN2 = float(np.log(2.0))

# aux column layout
A_E = 0       # E[d,j] = (d == hs+j)
A_N = 64      # NF = 126 + E
A_CN = 128    # ln2 * NF
A_OME = 192   # 1 - E
A_COLS = 256

# gat column layout (host-gathered raw values, need softplus on device)
G_TG = 0      # tg[d,h]  = t[d,h,hs+h]          (h-column of each slice)
G_TD = 64     # td[d,h]  = t[d,h,d]             (diagonal of each slice)
G_TRH = 128   # trh[m,h] = s_sib[b,hg,hg,m]     (row h of each slice)
G_COLS = 192

_PROGRAM = None


def _build_program():
    import concourse.bacc as bacc
    import concourse.mybir as mybir
    import concourse.tile as tile

    fp32 = mybir.dt.float32
    AF = mybir.ActivationFunctionType
    OP = mybir.AluOpType

    # Exp and Ln live in one PWP table; without this filter the table
    # chooser maps Exp to exp_and_others and Ln to natural_log_exp_and_
    # others and reloads the ACT table (~2.7us) between every pair.
    if not getattr(bacc, "_lbp_act_tables_patched", False):
        _orig_tables = bacc.get_activation_tables

        def _ln_exp_only(arch):
            t = _orig_tables(arch)
            # act_func_set_id is the dict index: keep order and size, only
            # drop Exp/Ln membership from every other set so the chooser
            # lands both on natural_log_exp_and_others.
            exp_ln = {AF.Exp, AF.Ln}
            return {
                name: (funcs if name == "natural_log_exp_and_others"
                       else set(funcs) - exp_ln)
                for name, funcs in t.items()
            }

        bacc.get_activation_tables = _ln_exp_only
        bacc._lbp_act_tables_patched = True

    nc = bacc.Bacc(None, target_bir_lowering=False)

    t_d = nc.dram_tensor("t", [L, H, L], fp32, kind="ExternalInput")
    se_d = nc.dram_tensor("se", [L, H, 2], fp32, kind="ExternalInput")
    gat_d = nc.dram_tensor("gat", [L, G_COLS], fp32, kind="ExternalInput")
    aux_d = nc.dram_tensor("aux", [L, A_COLS], fp32, kind="ExternalInput")
    o_d = nc.dram_tensor("o", [L, H, 2], fp32, kind="ExternalOutput")

    with tile.TileContext(nc) as tc:
        with (
            tc.tile_pool(name="const", bufs=1) as cpool,
            tc.tile_pool(name="stream", bufs=3) as spool,
            tc.tile_pool(name="spst", bufs=3) as sppool,
            tc.tile_pool(name="work", bufs=1) as wpool,
            tc.tile_pool(name="psum", bufs=1, space="PSUM") as ppool,
        ):
            aux = cpool.tile([L, A_COLS], fp32, tag="aux")
            se = cpool.tile([L, H, 2], fp32, tag="se")
            gat = cpool.tile([L, G_COLS], fp32, tag="gat")
            ones = cpool.tile([L, L], fp32, tag="ones")
            zb = cpool.tile([L, 1], fp32, tag="zb")
            ob = cpool.tile([L, 1], fp32, tag="ob")

            nc.sync.dma_start(gat[:], gat_d[:])
            nc.sync.dma_start(aux[:], aux_d[:])
            nc.sync.dma_start(se[:], se_d[:])
            nc.gpsimd.memset(ones[:], 1.0)
            nc.gpsimd.memset(zb[:], 0.0)
            nc.gpsimd.memset(ob[:], 1.0)

            E = aux[:, A_E:A_E + H]
            NF = aux[:, A_N:A_N + H]
            CN = aux[:, A_CN:A_CN + H]
            OME = aux[:, A_OME:A_OME + H]

            # softplus of the gathered side values: G | DG | ROWH
            gsp = wpool.tile([L, G_COLS], fp32, tag="gsp")
            nc.scalar.activation(gsp[:], gat[:], AF.Exp, bias=zb[:])
            nc.scalar.activation(gsp[:], gsp[:], AF.Ln, bias=ob[:])
            G = gsp[:, G_TG:G_TG + H]
            DG = gsp[:, G_TD:G_TD + H]
            ROWH = gsp[:, G_TRH:G_TRH + H]

            RS = wpool.tile([L, H], fp32, tag="RS")
            CSs = wpool.tile([L, H], fp32, tag="CSs")
            cs_ps = ppool.tile([L, H], fp32, tag="cs_ps")

            # stream the 4 MiB shard: exp -> ln(+1) -> row sums + col sums
            for ci in range(NCHUNK):
                tch = spool.tile([L, CH, L], fp32, tag="tch")
                nc.sync.dma_start(tch[:], t_d[:, ci * CH:(ci + 1) * CH, :])
                sp = sppool.tile([L, CH, L], fp32, tag="sp")
                nc.scalar.activation(sp[:], tch[:], AF.Exp, bias=zb[:])
                nc.scalar.activation(sp[:], sp[:], AF.Ln, bias=ob[:])
                nc.vector.tensor_reduce(
                    RS[:, ci * CH:(ci + 1) * CH], sp[:],
                    axis=mybir.AxisListType.X, op=OP.add,
                )
                for j in range(CH):
                    h = ci * CH + j
                    nc.tensor.matmul(
                        cs_ps[:, h:h + 1],
                        sp[:, j, :],
                        ones[:, 0:1],
                        start=True, stop=True,
                    )

            nc.vector.tensor_copy(CSs[:], cs_ps[:])

            # ---- batched [128, 64] tail algebra ----
            PD = wpool.tile([L, H], fp32, tag="PD")
            nc.vector.tensor_sub(PD[:], se[:, :, 1], se[:, :, 0])

            SPF = wpool.tile([L, H], fp32, tag="SPF")
            CPF = wpool.tile([L, H], fp32, tag="CPF")
            tA = wpool.tile([L, H], fp32, tag="tA")
            tB = wpool.tile([L, H], fp32, tag="tB")

            # SPF = RS - G - DG + E*G - CN
            nc.vector.tensor_sub(tA[:], RS[:], G[:])
            nc.vector.tensor_sub(tA[:], tA[:], DG[:])
            nc.vector.tensor_mul(tB[:], E, G[:])
            nc.vector.tensor_add(tA[:], tA[:], tB[:])
            nc.vector.tensor_sub(SPF[:], tA[:], CN)
            # CPF = CS - ROWH - DG + E*DG - CN
            nc.vector.tensor_sub(tA[:], CSs[:], ROWH)
            nc.vector.tensor_sub(tA[:], tA[:], DG[:])
            nc.vector.tensor_mul(tB[:], E, DG[:])
            nc.vector.tensor_add(tA[:], tA[:], tB[:])
            nc.vector.tensor_sub(CPF[:], tA[:], CN)

            D1 = wpool.tile([L, H], fp32, tag="D1")
            nc.vector.tensor_sub(D1[:], CPF[:], SPF[:])

            # r1 = PD + PD*NF + CPF
            r1 = wpool.tile([L, H], fp32, tag="r1")
            nc.vector.tensor_mul(tA[:], PD[:], NF)
            nc.vector.tensor_add(tA[:], tA[:], PD[:])
            nc.vector.tensor_add(r1[:], tA[:], CPF[:])

            # S0 = bcast(sum_s PD*(1-E))  via ones-stationary matmul
            bc0 = ppool.tile([L, H], fp32, tag="bc0")
            nc.vector.tensor_mul(tB[:], PD[:], OME)
            nc.tensor.matmul(bc0[:], ones[:], tB[:], start=True, stop=True)

            # r2 = r1*NF + 2*PD - E*PD - S0 + D1
            r2 = wpool.tile([L, H], fp32, tag="r2")
            nc.vector.tensor_mul(tA[:], r1[:], NF)
            nc.vector.scalar_tensor_tensor(
                tA[:], PD[:], 2.0, tA[:], op0=OP.mult, op1=OP.add)
            nc.vector.tensor_mul(tB[:], E, PD[:])
            nc.vector.tensor_sub(tA[:], tA[:], tB[:])
            nc.vector.tensor_sub(tA[:], tA[:], bc0[:])
            nc.vector.tensor_add(r2[:], tA[:], D1[:])

            # S1 = bcast(sum_s r1*(1-E))
            bc1 = ppool.tile([L, H], fp32, tag="bc1")
            nc.vector.tensor_mul(tB[:], r1[:], OME)
            nc.tensor.matmul(bc1[:], ones[:], tB[:], start=True, stop=True)

            # bdiff = (r2+PD)*NF + PD + r1 - E*r1 - S1 + CPF + D1
            bd = wpool.tile([L, H], fp32, tag="bd")
            nc.vector.tensor_add(tA[:], r2[:], PD[:])
            nc.vector.tensor_mul(tA[:], tA[:], NF)
            nc.vector.tensor_add(tA[:], tA[:], PD[:])
            nc.vector.tensor_add(tA[:], tA[:], r1[:])
            nc.vector.tensor_mul(tB[:], E, r1[:])
            nc.vector.tensor_sub(tA[:], tA[:], tB[:])
            nc.vector.tensor_sub(tA[:], tA[:], bc1[:])
            nc.vector.tensor_add(tA[:], tA[:], CPF[:])
            nc.vector.tensor_add(bd[:], tA[:], D1[:])

            # ---- stable sigmoid pair: m=max(bd,0); ei=exp(arg<=0) ----
            mx = wpool.tile([L, H], fp32, tag="mx")
            e1 = wpool.tile([L, H], fp32, tag="e1")
            e0 = wpool.tile([L, H], fp32, tag="e0")
            nc.vector.tensor_scalar_max(mx[:], bd[:], 0.0)
            nc.vector.tensor_sub(tA[:], bd[:], mx[:])
            nc.scalar.activation(e1[:], tA[:], AF.Exp, bias=zb[:])
            nc.scalar.activation(e0[:], mx[:], AF.Exp, bias=zb[:], scale=-1.0)

            osb = wpool.tile([L, H, 2], fp32, tag="osb")
            nc.vector.tensor_add(tA[:], e0[:], e1[:])
            nc.vector.reciprocal(tB[:], tA[:])
            nc.vector.tensor_mul(osb[:, :, 1], e1[:], tB[:])
            nc.vector.tensor_mul(osb[:, :, 0], e0[:], tB[:])
            nc.sync.dma_start(o_d[:], osb[:])

    nc.compile()
    return nc


def _core_inputs(s_edge, s_sib, c):
    b, hs = c >> 1, (c & 1) * H
    t = np.ascontiguousarray(s_sib[b, :, hs:hs + H, :], dtype=np.float32)
    se = np.ascontiguousarray(s_edge[b, :, hs:hs + H, :], dtype=np.float32)
    d = np.arange(L)
    hl = np.arange(H)
    gat = np.empty((L, G_COLS), dtype=np.float32)
    gat[:, G_TG:G_TG + H] = t[d[:, None], hl[None, :], (hs + hl)[None, :]]
    gat[:, G_TD:G_TD + H] = t[d[:, None], hl[None, :], d[:, None]]
    gat[:, G_TRH:G_TRH + H] = s_sib[
        b, (hs + hl)[None, :], (hs + hl)[None, :], d[:, None]]
    aux = np.zeros((L, A_COLS), dtype=np.float32)
    E = (d[:, None] == (hs + hl)[None, :]).astype(np.float32)
    aux[:, A_E:A_E + H] = E
    aux[:, A_N:A_N + H] = 126.0 + E
    aux[:, A_CN:A_CN + H] = LN2 * (126.0 + E)
    aux[:, A_OME:A_OME + H] = 1.0 - E
    return {"t": t, "se": se, "gat": gat, "aux": aux}


def make_in_maps(s_edge, s_sib):
    return [_core_inputs(s_edge, s_sib, c) for c in range(N_CORES)]


def get_program():
    global _PROGRAM
    if _PROGRAM is None:
        _PROGRAM = _build_program()
    return _PROGRAM


def assemble(results):
    out = np.empty((4, L, L, 2), dtype=np.float32)
    for c in range(N_CORES):
        b, hs = c >> 1, (c & 1) * H
        out[b, :, hs:hs + H, :] = results[c]["o"].reshape(L, H, 2)
    return out


def kernel(s_edge, s_sib, mask):
    from concourse.bass_utils import run_bass_kernel_spmd

    s_edge = np.asarray(s_edge)
    s_sib = np.asarray(s_sib)
    mask = np.asarray(mask)
    assert mask.all(), "kernel specialized for the spec's all-ones mask"

    nc = get_program()
    in_maps = make_in_maps(s_edge, s_sib)
    res = run_bass_kernel_spmd(nc, in_maps, list(range(N_CORES))).results
    return assemble(res)



# revision 2
# speedup vs baseline: 1.1525x; 1.1525x over previous
"""Trainium2 Bass kernel for LoopyBeliefPropagation (3-iter, mask=ones).

Math: for each (b, h) slice define tile[d,s] = s_sib[b,d,h,s],
SP = softplus(tile), F[d,s] = (s!=h)(s!=d), Pdiff[d] =
s_edge[b,d,h,1] - s_edge[b,d,h,0].  Tracking only the message channel
difference delta = m_sib[...,1] - m_sib[...,0] collapses the reference's
3-iteration loop into closed form (see kernel_baseline.py for the full
derivation):

  SPF = RS - K1, CPF = CS - K2, D1 = CPF - SPF
  r1  = PDN + CPF
  r2  = r1*NF + Q2 + D1
  bdiff = (r2+PD)*NF + r1*OME - S1 + (2*CPF - SPF + PD)
  out[b,d,h,1] = sigmoid(bdiff), out[b,d,h,0] = sigmoid(-bdiff)

where RS[d,h] = sum_s SP[d,s] (device row-reduce of the softplus
stream), CS[d,h] = sum_s SP[s,d] (device column sums via ones-column
matmuls), S1 = colsum(r1*OME) broadcast (device matmul), and K1, K2,
NF, OME, PDN, Q2, PD are stream-independent [L,H] constants folded on
the host (they only touch O(L*H) gathered values, not the 4 MiB
stream).  softplus = Ln(Exp(x) + 1) using the natural_log_exp ACT
table; the +1 rides the Ln bias.

Device schedule: the s_sib shard streams in ramped h-chunks (small
first chunk so ACT starts early, small last chunk so the final DVE
row-reduce is short); Exp/Ln run back-to-back on ACT (the spine),
row-reduces on DVE and per-h column-sum matmuls on PE trail each Ln.

Sharding: 8 cores x (b in 0..3, h-half in {0:64, 64:128}).
"""

import numpy as np

L = 128
H = 64            # h-slices per core
N_CORES = 8
LN2 = float(np.log(2.0))

# ramped chunk sizes (h-slices per streamed chunk)
CHUNKS = [2, 4, 8, 16, 20, 10, 4]
assert sum(CHUNKS) == H

# host-constant column layout: 7 tensors of H columns each
C_K1 = 0 * H
C_K2 = 1 * H
C_NF = 2 * H
C_OME = 3 * H
C_PDN = 4 * H
C_Q2 = 5 * H
C_PD = 6 * H
C_COLS = 7 * H

_PROGRAM = None


def _build_program():
    import concourse.bacc as bacc
    import concourse.mybir as mybir
    import concourse.tile as tile

    fp32 = mybir.dt.float32
    AF = mybir.ActivationFunctionType
    OP = mybir.AluOpType

    # Exp and Ln live in one PWP table; without this filter the table
    # chooser maps Exp to exp_and_others and Ln to natural_log_exp_and_
    # others and reloads the ACT table (~1.3us) between every pair.
    if not getattr(bacc, "_lbp_act_tables_patched", False):
        _orig_tables = bacc.get_activation_tables

        def _ln_exp_only(arch):
            t = _orig_tables(arch)
            exp_ln = {AF.Exp, AF.Ln}
            return {
                name: (funcs if name == "natural_log_exp_and_others"
                       else set(funcs) - exp_ln)
                for name, funcs in t.items()
            }

        bacc.get_activation_tables = _ln_exp_only
        bacc._lbp_act_tables_patched = True

    nc = bacc.Bacc(None, target_bir_lowering=False)

    t_d = nc.dram_tensor("t", [L, H, L], fp32, kind="ExternalInput")
    hx_d = nc.dram_tensor("hx", [L, C_COLS], fp32, kind="ExternalInput")
    o_d = nc.dram_tensor("o", [L, H, 2], fp32, kind="ExternalOutput")

    with tile.TileContext(nc) as tc:
        with (
            tc.tile_pool(name="const", bufs=1) as cpool,
            tc.tile_pool(name="stream", bufs=3) as spool,
            tc.tile_pool(name="spst", bufs=3) as sppool,
            tc.tile_pool(name="work", bufs=1) as wpool,
            tc.tile_pool(name="psum", bufs=1, space="PSUM") as ppool,
        ):
            hx = cpool.tile([L, C_COLS], fp32, tag="hx")
            ones = cpool.tile([L, L], fp32, tag="ones")
            zb = cpool.tile([L, 1], fp32, tag="zb")
            ob = cpool.tile([L, 1], fp32, tag="ob")

            RS = wpool.tile([L, H], fp32, tag="RS")
            cs_ps = ppool.tile([L, H], fp32, tag="cs_ps")

            # stream chunks first on the DMA queue: the first chunk's
            # transfer is the critical-path opener for the ACT spine.
            tchs = []
            h0 = 0
            for ci, ch in enumerate(CHUNKS):
                tch = spool.tile([L, ch, L], fp32, tag=f"tch{ci}")
                nc.sync.dma_start(tch[:], t_d[:, h0:h0 + ch, :])
                tchs.append((tch, h0, ch))
                if ci == 0:
                    # host constants ride the queue right behind chunk 0
                    nc.sync.dma_start(hx[:], hx_d[:])
                h0 += ch

            nc.gpsimd.memset(ones[:], 1.0)
            nc.gpsimd.memset(zb[:], 0.0)
            nc.gpsimd.memset(ob[:], 1.0)

            K1 = hx[:, C_K1:C_K1 + H]
            K2 = hx[:, C_K2:C_K2 + H]
            NF = hx[:, C_NF:C_NF + H]
            OME = hx[:, C_OME:C_OME + H]
            PDN = hx[:, C_PDN:C_PDN + H]
            Q2 = hx[:, C_Q2:C_Q2 + H]
            PD = hx[:, C_PD:C_PD + H]

            # spine: exp -> ln(+1) per chunk; row sums + col sums trail
            for ci, (tch, h0, ch) in enumerate(tchs):
                sp = sppool.tile([L, ch, L], fp32, tag=f"sp{ci}")
                nc.scalar.activation(sp[:], tch[:], AF.Exp, bias=zb[:])
                nc.scalar.activation(sp[:], sp[:], AF.Ln, bias=ob[:])
                nc.vector.tensor_reduce(
                    RS[:, h0:h0 + ch], sp[:],
                    axis=mybir.AxisListType.X, op=OP.add,
                )
                for j in range(ch):
                    h = h0 + j
                    nc.tensor.matmul(
                        cs_ps[:, h:h + 1],
                        sp[:, j, :],
                        ones[:, 0:1],
                        start=True, stop=True,
                    )

            # ---- batched [128, 64] tail algebra ----
            SPF = wpool.tile([L, H], fp32, tag="SPF")
            CPF = wpool.tile([L, H], fp32, tag="CPF")
            r1 = wpool.tile([L, H], fp32, tag="r1")
            w = wpool.tile([L, H], fp32, tag="w")
            tA = wpool.tile([L, H], fp32, tag="tA")
            tB = wpool.tile([L, H], fp32, tag="tB")
            u1 = wpool.tile([L, H], fp32, tag="u1")
            bd = wpool.tile([L, H], fp32, tag="bd")

            nc.vector.tensor_sub(SPF[:], RS[:], K1)
            nc.vector.tensor_sub(CPF[:], cs_ps[:], K2)
            nc.vector.tensor_add(r1[:], PDN, CPF[:])
            nc.vector.tensor_mul(w[:], r1[:], OME)

            bc1 = ppool.tile([L, H], fp32, tag="bc1")
            nc.tensor.matmul(bc1[:], ones[:], w[:], start=True, stop=True)

            # r2 = r1*NF + Q2 + D1   (D1 = CPF - SPF)
            nc.vector.tensor_mul(tA[:], r1[:], NF)
            nc.vector.tensor_add(tA[:], tA[:], Q2)
            nc.vector.tensor_sub(tB[:], CPF[:], SPF[:])
            nc.vector.tensor_add(tA[:], tA[:], tB[:])
            # bdiff = (r2+PD)*NF + w - S1 + (2*CPF - SPF + PD)
            nc.vector.tensor_add(tA[:], tA[:], PD)
            nc.vector.tensor_mul(tA[:], tA[:], NF)
            nc.vector.tensor_add(tA[:], tA[:], w[:])
            nc.vector.tensor_sub(tA[:], tA[:], bc1[:])
            nc.vector.scalar_tensor_tensor(
                u1[:], CPF[:], 2.0, SPF[:], op0=OP.mult, op1=OP.subtract)
            nc.vector.tensor_add(u1[:], u1[:], PD)
            nc.vector.tensor_add(bd[:], tA[:], u1[:])

            # ---- stable sigmoid pair via one fused Exp ----
            # big = [bd - mx | -mx]; eb = exp(big); out = eb / rowpair-sum
            mx = wpool.tile([L, H], fp32, tag="mx")
            big = wpool.tile([L, 2 * H], fp32, tag="big")
            nc.vector.tensor_scalar_max(mx[:], bd[:], 0.0)
            nc.vector.tensor_sub(big[:, 0:H], bd[:], mx[:])
            nc.vector.scalar_tensor_tensor(
                big[:, H:2 * H], mx[:], -1.0, mx[:],
                op0=OP.mult, op1=OP.bypass)
            eb = wpool.tile([L, 2 * H], fp32, tag="eb")
            nc.scalar.activation(eb[:], big[:], AF.Exp, bias=zb[:])

            osb = wpool.tile([L, H, 2], fp32, tag="osb")
            nc.vector.tensor_add(tA[:], eb[:, 0:H], eb[:, H:2 * H])
            nc.vector.reciprocal(tB[:], tA[:])
            nc.vector.tensor_mul(osb[:, :, 1], eb[:, 0:H], tB[:])
            nc.vector.tensor_mul(osb[:, :, 0], eb[:, H:2 * H], tB[:])
            nc.sync.dma_start(o_d[:], osb[:])

    nc.compile()
    return nc


def _softplus(x):
    return np.logaddexp(0.0, x)


def _core_inputs(s_edge, s_sib, c):
    b, hs = c >> 1, (c & 1) * H
    t = np.ascontiguousarray(s_sib[b, :, hs:hs + H, :], dtype=np.float32)
    d = np.arange(L)
    hl = np.arange(H)

    # host-gathered softplus side values
    G = _softplus(t[d[:, None], hl[None, :], (hs + hl)[None, :]])
    DG = _softplus(t[d[:, None], hl[None, :], d[:, None]])
    ROWH = _softplus(
        s_sib[b, (hs + hl)[None, :], (hs + hl)[None, :], d[:, None]])

    E = (d[:, None] == (hs + hl)[None, :]).astype(np.float32)
    NF = 126.0 + E
    CN = LN2 * NF
    OME = 1.0 - E
    PD = (s_edge[b, :, hs:hs + H, 1]
          - s_edge[b, :, hs:hs + H, 0]).astype(np.float32)

    K1 = G + DG - E * G + CN
    K2 = ROWH + DG - E * DG + CN
    PDN = PD * (NF + 1.0)
    S0 = np.sum(PD * OME, axis=0, keepdims=True)   # [1, H] col sums
    Q2 = 2.0 * PD - E * PD - S0                    # broadcast over d

    hx = np.empty((L, C_COLS), dtype=np.float32)
    hx[:, C_K1:C_K1 + H] = K1
    hx[:, C_K2:C_K2 + H] = K2
    hx[:, C_NF:C_NF + H] = NF
    hx[:, C_OME:C_OME + H] = OME
    hx[:, C_PDN:C_PDN + H] = PDN
    hx[:, C_Q2:C_Q2 + H] = Q2
    hx[:, C_PD:C_PD + H] = PD
    return {"t": t, "hx": hx}


def make_in_maps(s_edge, s_sib):
    return [_core_inputs(s_edge, s_sib, c) for c in range(N_CORES)]


def get_program():
    global _PROGRAM
    if _PROGRAM is None:
        _PROGRAM = _build_program()
    return _PROGRAM


def assemble(results):
    out = np.empty((4, L, L, 2), dtype=np.float32)
    for c in range(N_CORES):
        b, hs = c >> 1, (c & 1) * H
        out[b, :, hs:hs + H, :] = results[c]["o"].reshape(L, H, 2)
    return out


def kernel(s_edge, s_sib, mask):
    from concourse.bass_utils import run_bass_kernel_spmd

    s_edge = np.asarray(s_edge)
    s_sib = np.asarray(s_sib)
    mask = np.asarray(mask)
    assert mask.all(), "kernel specialized for the spec's all-ones mask"

    nc = get_program()
    in_maps = make_in_maps(s_edge, s_sib)
    res = run_bass_kernel_spmd(nc, in_maps, list(range(N_CORES))).results
    return assemble(res)


# revision 4
# speedup vs baseline: 1.2183x; 1.0571x over previous
"""Trainium2 Bass kernel for LoopyBeliefPropagation (3-iter, mask=ones).

Math: for each (b, h) slice define tile[d,s] = s_sib[b,d,h,s],
SP = softplus(tile).  Collapsing the reference's 3-iteration loop into
closed form (see kernel_baseline.py) and folding every
stream-independent term into host constants leaves, per (d,h):

  RS[d,h] = sum_s SP[d,s]        (device row-reduce of the stream)
  CS[d,h] = sum_s SP[s,d]        (device col sums via ones matmuls)
  r1 = CS + C5
  w  = r1 * OME,  S1 = colsum(w) broadcast   (device matmul)
  bdiff = r1*NF2 + (CS + C7)*NFp2 - RS*NFp1 + w - S1
  out1 = sigmoid(bdiff) = 1/(1+q),  q = exp(-max(bdiff, -30))
  out0 = q * out1

C5, C7, NF2, NFp1, NFp2, OME are [L,H] host constants (they only touch
O(L*H) gathered values, not the 4 MiB stream).  softplus = Ln(Exp(x)+1)
using the natural_log_exp ACT table; the +1 rides the Ln bias.

Device schedule: the s_sib shard streams in ramped h-chunks; Exp/Ln run
back-to-back on ACT (the spine; a dummy Exp right after a memset pulls
the ACT table load to t~0).  Row-reduces (DVE) and col-sum matmuls (PE)
trail each Ln.  The h-columns are fully independent, so the tail
algebra runs in three h-slices: the first two overlap the spine, only
the last ~16-wide slice (plus its output DMA) sits past the spine.

Sharding: 8 cores x (b in 0..3, h-half in {0:64, 64:128}).
"""

import numpy as np

L = 128
H = 64            # h-slices per core
N_CORES = 8
LN2 = float(np.log(2.0))

# ramped chunk sizes; tails cover h [0:32], [32:48], [48:64]
CHUNKS = [2, 4, 10, 16, 16, 10, 6]
TAILS = [(0, 32, 3), (32, 16, 4), (48, 16, 6)]   # (h0, width, after-chunk)
assert sum(CHUNKS) == H

# host-constant column layout: 6 tensors of H columns each
C_C5 = 0 * H
C_C7 = 1 * H
C_NF2 = 2 * H
C_NFP1 = 3 * H
C_NFP2 = 4 * H
C_OME = 5 * H
C_COLS = 6 * H

_PROGRAM = None


def _build_program():
    import concourse.bacc as bacc
    import concourse.mybir as mybir
    import concourse.tile as tile

    fp32 = mybir.dt.float32
    AF = mybir.ActivationFunctionType
    OP = mybir.AluOpType

    # Exp and Ln live in one PWP table; without this filter the table
    # chooser maps Exp to exp_and_others and Ln to natural_log_exp_and_
    # others and reloads the ACT table (~1.3us) between every pair.
    if not getattr(bacc, "_lbp_act_tables_patched", False):
        _orig_tables = bacc.get_activation_tables

        def _ln_exp_only(arch):
            t = _orig_tables(arch)
            exp_ln = {AF.Exp, AF.Ln}
            return {
                name: (funcs if name == "natural_log_exp_and_others"
                       else set(funcs) - exp_ln)
                for name, funcs in t.items()
            }

        bacc.get_activation_tables = _ln_exp_only
        bacc._lbp_act_tables_patched = True

    nc = bacc.Bacc(None, target_bir_lowering=False)

    t_d = nc.dram_tensor("t", [L, H, L], fp32, kind="ExternalInput")
    hx_d = nc.dram_tensor("hx", [L, C_COLS], fp32, kind="ExternalInput")
    o_d = nc.dram_tensor("o", [L, H, 2], fp32, kind="ExternalOutput")

    with tile.TileContext(nc) as tc:
        with (
            tc.tile_pool(name="work", bufs=1) as wp,
            tc.tile_pool(name="psum", bufs=1, space="PSUM") as pp,
        ):
            zb = wp.tile([L, 1], fp32, tag="zb")
            ob = wp.tile([L, 1], fp32, tag="ob")
            ones = wp.tile([L, L], fp32, tag="ones")
            nc.gpsimd.memset(zb[:], 0.0)
            nc.gpsimd.memset(ob[:], 1.0)
            nc.gpsimd.memset(ones[:], 1.0)

            # dummy table-user: Bacc places the ACT table load right
            # before this, so it runs at t~0 instead of after chunk 0.
            dummy = wp.tile([L, 1], fp32, tag="dummy")
            nc.scalar.activation(dummy[:], zb[:], AF.Exp, bias=zb[:])

            # stream DMAs: chunk 0 first (critical-path opener); host
            # constants ride the queue behind chunk 2.
            hx = wp.tile([L, C_COLS], fp32, tag="hx")
            tchs = []
            h0 = 0
            for ci, ch in enumerate(CHUNKS):
                tch = wp.tile([L, ch, L], fp32, tag=f"tch{ci}")
                nc.sync.dma_start(tch[:], t_d[:, h0:h0 + ch, :])
                tchs.append((tch, h0, ch))
                if ci == 2:
                    nc.sync.dma_start(hx[:], hx_d[:])
                h0 += ch

            # per-tail state
            tails = []
            for ti, (th0, tw, _) in enumerate(TAILS):
                tails.append({
                    "RS": wp.tile([L, tw], fp32, tag=f"RS{ti}", name=f"RS{ti}"),
                    "cs": pp.tile([L, tw], fp32, tag=f"cs{ti}", name=f"cs{ti}"),
                    "bc": pp.tile([L, tw], fp32, tag=f"bc{ti}", name=f"bc{ti}"),
                    "osb": wp.tile([L, tw, 2], fp32, tag=f"osb{ti}", name=f"osb{ti}"),
                })

            def tail_of(h):
                for ti, (th0, tw, _) in enumerate(TAILS):
                    if th0 <= h < th0 + tw:
                        return ti, h - th0
                raise AssertionError

            def tail_chain(ti):
                """DVE algebra for tail slice ti, up to bdc (exp input)."""
                th0, tw, _ = TAILS[ti]
                T = tails[ti]
                sl = slice(th0, th0 + tw)
                r1 = wp.tile([L, tw], fp32, tag=f"r1_{ti}")
                w_ = wp.tile([L, tw], fp32, tag=f"w_{ti}")
                a = wp.tile([L, tw], fp32, tag=f"a_{ti}")
                csc = wp.tile([L, tw], fp32, tag=f"csc_{ti}")
                b = wp.tile([L, tw], fp32, tag=f"b_{ti}")
                rsn = wp.tile([L, tw], fp32, tag=f"rsn_{ti}")
                bd = wp.tile([L, tw], fp32, tag=f"bd_{ti}")
                bdc = wp.tile([L, tw], fp32, tag=f"bdc_{ti}")
                T["bdc"] = bdc
                T["q"] = wp.tile([L, tw], fp32, tag=f"q_{ti}", name=f"q_{ti}")
                T["s"] = wp.tile([L, tw], fp32, tag=f"s_{ti}", name=f"s_{ti}")
                nc.vector.tensor_add(r1[:], T["cs"][:], hx[:, C_C5 + th0:C_C5 + th0 + tw])
                nc.vector.tensor_mul(w_[:], r1[:], hx[:, C_OME + th0:C_OME + th0 + tw])
                nc.tensor.matmul(T["bc"][:], ones[:], w_[:], start=True, stop=True)
                nc.vector.tensor_mul(a[:], r1[:], hx[:, C_NF2 + th0:C_NF2 + th0 + tw])
                nc.vector.tensor_add(csc[:], T["cs"][:], hx[:, C_C7 + th0:C_C7 + th0 + tw])
                nc.vector.tensor_mul(b[:], csc[:], hx[:, C_NFP2 + th0:C_NFP2 + th0 + tw])
                nc.vector.tensor_mul(rsn[:], T["RS"][:], hx[:, C_NFP1 + th0:C_NFP1 + th0 + tw])
                nc.vector.tensor_add(bd[:], a[:], b[:])
                nc.vector.tensor_sub(bd[:], bd[:], rsn[:])
                nc.vector.tensor_add(bd[:], bd[:], w_[:])
                nc.vector.tensor_sub(bd[:], bd[:], T["bc"][:])
                nc.vector.tensor_scalar_max(bdc[:], bd[:], -30.0)

            def tail_exp(ti):
                T = tails[ti]
                nc.scalar.activation(T["q"][:], T["bdc"][:], AF.Exp,
                                     bias=zb[:], scale=-1.0)

            def tail_final(ti):
                T = tails[ti]
                nc.vector.tensor_scalar_add(T["s"][:], T["q"][:], 1.0)
                nc.vector.reciprocal(T["osb"][:, :, 1], T["s"][:])
                nc.vector.tensor_mul(T["osb"][:, :, 0], T["q"][:],
                                     T["osb"][:, :, 1])

            # spine: exp -> ln(+1) per chunk; row sums + col sums trail
            rs_done = 0
            for ci, (tch, h0, ch) in enumerate(tchs):
                sp = wp.tile([L, ch, L], fp32, tag=f"sp{ci}")
                nc.scalar.activation(sp[:], tch[:], AF.Exp, bias=zb[:])
                nc.scalar.activation(sp[:], sp[:], AF.Ln, bias=ob[:])
                ti, off = tail_of(h0)
                assert tail_of(h0 + ch - 1)[0] == ti  # chunk within one tail
                T = tails[ti]
                nc.vector.tensor_reduce(
                    T["RS"][:, off:off + ch], sp[:],
                    axis=mybir.AxisListType.X, op=OP.add,
                )
                for j in range(ch):
                    nc.tensor.matmul(
                        T["cs"][:, off + j:off + j + 1],
                        sp[:, j, :],
                        ones[:, 0:1],
                        start=True, stop=True,
                    )
                # emit mid-stream tail chains as their inputs complete
                for ti2, (th0, tw, after) in enumerate(TAILS[:2]):
                    if after == ci:
                        tail_chain(ti2)

            tail_chain(2)
            for ti in range(3):
                tail_exp(ti)
            for ti in range(3):
                tail_final(ti)
            for ti, (th0, tw, _) in enumerate(TAILS):
                nc.sync.dma_start(o_d[:, th0:th0 + tw, :], tails[ti]["osb"][:])

    nc.compile()
    return nc


def _softplus(x):
    return np.logaddexp(0.0, x)


def _core_inputs(s_edge, s_sib, c):
    b, hs = c >> 1, (c & 1) * H
    t = np.ascontiguousarray(s_sib[b, :, hs:hs + H, :], dtype=np.float32)
    d = np.arange(L)
    hl = np.arange(H)

    G = _softplus(t[d[:, None], hl[None, :], (hs + hl)[None, :]])
    DG = _softplus(t[d[:, None], hl[None, :], d[:, None]])
    ROWH = _softplus(
        s_sib[b, (hs + hl)[None, :], (hs + hl)[None, :], d[:, None]])

    E = (d[:, None] == (hs + hl)[None, :]).astype(np.float32)
    NF = 126.0 + E
    CN = LN2 * NF
    OME = 1.0 - E
    PD = (s_edge[b, :, hs:hs + H, 1]
          - s_edge[b, :, hs:hs + H, 0]).astype(np.float32)

    K1 = G + DG - E * G + CN
    K2 = ROWH + DG - E * DG + CN
    PDN = PD * (NF + 1.0)
    S0 = np.sum(PD * OME, axis=0, keepdims=True)   # [1, H] col sums
    Q2 = 2.0 * PD - E * PD - S0
    C6 = (Q2 + PD) * NF + (K1 - K2) * NF + K1 - 2.0 * K2 + PD

    hx = np.empty((L, C_COLS), dtype=np.float32)
    hx[:, C_C5:C_C5 + H] = PDN - K2
    hx[:, C_C7:C_C7 + H] = C6 / (NF + 2.0)
    hx[:, C_NF2:C_NF2 + H] = NF * NF
    hx[:, C_NFP1:C_NFP1 + H] = NF + 1.0
    hx[:, C_NFP2:C_NFP2 + H] = NF + 2.0
    hx[:, C_OME:C_OME + H] = OME
    return {"t": t, "hx": hx}


def make_in_maps(s_edge, s_sib):
    return [_core_inputs(s_edge, s_sib, c) for c in range(N_CORES)]


def get_program():
    global _PROGRAM
    if _PROGRAM is None:
        _PROGRAM = _build_program()
    return _PROGRAM


def assemble(results):
    out = np.empty((4, L, L, 2), dtype=np.float32)
    for c in range(N_CORES):
        b, hs = c >> 1, (c & 1) * H
        out[b, :, hs:hs + H, :] = results[c]["o"].reshape(L, H, 2)
    return out


def kernel(s_edge, s_sib, mask):
    from concourse.bass_utils import run_bass_kernel_spmd

    s_edge = np.asarray(s_edge)
    s_sib = np.asarray(s_sib)
    mask = np.asarray(mask)
    assert mask.all(), "kernel specialized for the spec's all-ones mask"

    nc = get_program()
    in_maps = make_in_maps(s_edge, s_sib)
    res = run_bass_kernel_spmd(nc, in_maps, list(range(N_CORES))).results
    return assemble(res)


# revision 7
# speedup vs baseline: 1.2442x; 1.0213x over previous
"""Trainium2 Bass kernel for LoopyBeliefPropagation (3-iter, mask=ones).

Math: for each (b, h) slice define tile[d,s] = s_sib[b,d,h,s],
SP = softplus(tile).  Collapsing the reference's 3-iteration loop into
closed form (see kernel_baseline.py) and folding every
stream-independent term into host constants leaves, per (d,h):

  RS[d,h] = sum_s SP[d,s]        (row-reduce of the stream, DVE/Pool)
  CS[d,h] = sum_s SP[s,d]        (col sums via ones matmuls, PE)
  r1 = CS + C5
  w  = r1 * OME,  S1 = colsum(w) broadcast   (PE matmul)
  bdiff = r1*NF2 + (CS + C7)*NFp2 - RS*NFp1 + w - S1
  out1 = sigmoid(bdiff) = 1/(1+q),  q = exp(-max(bdiff, -30))
  out0 = q * out1

C5, C7, NF2, NFp1, NFp2, OME are [L,H] host constants (they only touch
O(L*H) gathered values, not the 4 MiB stream).  softplus = Ln(Exp(x)+1)
using the natural_log_exp ACT table; the +1 rides the Ln bias.

Device schedule: the s_sib shard streams in ramped h-chunks; Exp/Ln run
back-to-back on ACT (the spine; a dummy Exp right after a memset pulls
the ACT table load to t~0).  The h-columns are independent, so the tail
algebra runs in three h-slices pipelined against the spine; work is
spread over DVE and the otherwise-idle GPSIMD (Pool) engine so only the
last narrow slice (plus its output DMA) trails the spine.

Sharding: 8 cores x (b in 0..3, h-half in {0:64, 64:128}).
"""

import numpy as np

L = 128
H = 64            # h-slices per core
N_CORES = 8
LN2 = float(np.log(2.0))

# ramped chunk sizes; tails cover h [0:32], [32:58], [58:64]
CHUNKS = [2, 6, 10, 14, 16, 10, 4, 2]
TAILS = [(0, 32, 3), (32, 26, 5), (58, 6, 7)]    # (h0, width, after-chunk)
RED_ENG = ["v", "v", "v", "v", "v", "v", "v", "v"]   # per-chunk row-reduce
CHAIN_ENG = ["p", "p", "v"]                          # per-tail algebra
assert sum(CHUNKS) == H

# host-constant column layout: 6 tensors of H columns each
C_C5 = 0 * H
C_C7 = 1 * H
C_NF2 = 2 * H
C_NFP1 = 3 * H
C_NFP2 = 4 * H
C_OME = 5 * H
C_COLS = 6 * H

_PROGRAM = None


def _build_program():
    import concourse.bacc as bacc
    import concourse.mybir as mybir
    import concourse.tile as tile

    fp32 = mybir.dt.float32
    AF = mybir.ActivationFunctionType
    OP = mybir.AluOpType

    # Exp and Ln live in one PWP table; without this filter the table
    # chooser maps Exp to exp_and_others and Ln to natural_log_exp_and_
    # others and reloads the ACT table (~1.3us) between every pair.
    if not getattr(bacc, "_lbp_act_tables_patched", False):
        _orig_tables = bacc.get_activation_tables

        def _ln_exp_only(arch):
            t = _orig_tables(arch)
            exp_ln = {AF.Exp, AF.Ln}
            return {
                name: (funcs if name == "natural_log_exp_and_others"
                       else set(funcs) - exp_ln)
                for name, funcs in t.items()
            }

        bacc.get_activation_tables = _ln_exp_only
        bacc._lbp_act_tables_patched = True

    nc = bacc.Bacc(None, target_bir_lowering=False)

    t_d = nc.dram_tensor("t", [L, H, L], fp32, kind="ExternalInput")
    hx_d = nc.dram_tensor("hx", [L, C_COLS], fp32, kind="ExternalInput")
    o_d = nc.dram_tensor("o", [L, H, 2], fp32, kind="ExternalOutput")

    with tile.TileContext(nc) as tc:
        with (
            tc.tile_pool(name="work", bufs=1) as wp,
            tc.tile_pool(name="psum", bufs=1, space="PSUM") as pp,
        ):
            zb = wp.tile([L, 1], fp32, tag="zb")
            ob = wp.tile([L, 1], fp32, tag="ob")
            ones = wp.tile([L, L], fp32, tag="ones")
            nc.gpsimd.memset(zb[:], 0.0)
            nc.gpsimd.memset(ob[:], 1.0)
            nc.gpsimd.memset(ones[:], 1.0)

            # dummy table-user: Bacc places the ACT table load right
            # before this, so it runs at t~0 instead of after chunk 0.
            dummy = wp.tile([L, 1], fp32, tag="dummy")
            nc.scalar.activation(dummy[:], zb[:], AF.Exp, bias=zb[:])

            # stream DMAs: chunk 0 first (critical-path opener); host
            # constants ride the queue behind chunk 3.
            hx = wp.tile([L, C_COLS], fp32, tag="hx")
            tchs = []
            h0 = 0
            for ci, ch in enumerate(CHUNKS):
                tch = wp.tile([L, ch, L], fp32, tag=f"tch{ci}")
                nc.sync.dma_start(tch[:], t_d[:, h0:h0 + ch, :])
                tchs.append((tch, h0, ch))
                if ci == 3:
                    nc.sync.dma_start(hx[:], hx_d[:])
                h0 += ch

            # per-tail state
            tails = []
            for ti, (th0, tw, _) in enumerate(TAILS):
                tails.append({
                    "RS": wp.tile([L, tw], fp32, tag=f"RS{ti}", name=f"RS{ti}"),
                    "cs": pp.tile([L, tw], fp32, tag=f"cs{ti}", name=f"cs{ti}"),
                    "bc": pp.tile([L, tw], fp32, tag=f"bc{ti}", name=f"bc{ti}"),
                    "osb": wp.tile([L, tw, 2], fp32, tag=f"osb{ti}",
                                   name=f"osb{ti}"),
                })

            def tail_of(h):
                for ti, (th0, tw, _) in enumerate(TAILS):
                    if th0 <= h < th0 + tw:
                        return ti, h - th0
                raise AssertionError

            def hxs(base, ti):
                th0, tw, _ = TAILS[ti]
                return hx[:, base + th0:base + th0 + tw]

            def tail_chain_pre(ti, eng, cs_src):
                """Algebra for tail slice ti up to bd-partial (pre -S1).

                cs_src: SBUF copy of CS for Pool (it cannot read PSUM);
                the PSUM-reading finish lives in tail_chain_fin.
                """
                th0, tw, _ = TAILS[ti]
                T = tails[ti]
                r1 = wp.tile([L, tw], fp32, tag=f"r1_{ti}", name=f"r1_{ti}")
                w_ = wp.tile([L, tw], fp32, tag=f"w_{ti}", name=f"w_{ti}")
                a = wp.tile([L, tw], fp32, tag=f"a_{ti}", name=f"a_{ti}")
                csc = wp.tile([L, tw], fp32, tag=f"csc_{ti}", name=f"csc_{ti}")
                b = wp.tile([L, tw], fp32, tag=f"b_{ti}", name=f"b_{ti}")
                rsn = wp.tile([L, tw], fp32, tag=f"rsn_{ti}", name=f"rsn_{ti}")
                bd = wp.tile([L, tw], fp32, tag=f"bd_{ti}", name=f"bd_{ti}")
                T["bd"] = bd
                T["bdc"] = wp.tile([L, tw], fp32, tag=f"bdc_{ti}",
                                   name=f"bdc_{ti}")
                T["q"] = wp.tile([L, tw], fp32, tag=f"q_{ti}", name=f"q_{ti}")
                T["s"] = wp.tile([L, tw], fp32, tag=f"s_{ti}", name=f"s_{ti}")
                eng.tensor_add(r1[:], cs_src[:], hxs(C_C5, ti))
                eng.tensor_mul(w_[:], r1[:], hxs(C_OME, ti))
                nc.tensor.matmul(T["bc"][:], ones[:], w_[:],
                                 start=True, stop=True)
                eng.tensor_mul(a[:], r1[:], hxs(C_NF2, ti))
                eng.tensor_add(csc[:], cs_src[:], hxs(C_C7, ti))
                eng.tensor_mul(b[:], csc[:], hxs(C_NFP2, ti))
                eng.tensor_mul(rsn[:], T["RS"][:], hxs(C_NFP1, ti))
                eng.tensor_add(bd[:], a[:], b[:])
                eng.tensor_sub(bd[:], bd[:], rsn[:])
                eng.tensor_add(bd[:], bd[:], w_[:])

            def tail_chain_fin(ti):
                """DVE finish: -S1 (PSUM read) and the exp-domain clamp."""
                T = tails[ti]
                nc.vector.tensor_sub(T["bdc"][:], T["bd"][:], T["bc"][:])
                nc.vector.tensor_scalar_max(T["bdc"][:], T["bdc"][:], -30.0)

            def tail_chain(ti, eng):
                if eng is nc.vector:
                    tail_chain_pre(ti, eng, tails[ti]["cs"])
                else:
                    th0, tw, _ = TAILS[ti]
                    cs_s = wp.tile([L, tw], fp32, tag=f"cs_s{ti}",
                                   name=f"cs_s{ti}")
                    nc.vector.tensor_copy(cs_s[:], tails[ti]["cs"][:])
                    tail_chain_pre(ti, eng, cs_s)

            def tail_exp(ti):
                T = tails[ti]
                nc.scalar.activation(T["q"][:], T["bdc"][:], AF.Exp,
                                     bias=zb[:], scale=-1.0)

            def tail_final(ti):
                T = tails[ti]
                nc.vector.tensor_scalar_add(T["s"][:], T["q"][:], 1.0)
                nc.vector.reciprocal(T["osb"][:, :, 1], T["s"][:])
                nc.vector.tensor_mul(T["osb"][:, :, 0], T["q"][:],
                                     T["osb"][:, :, 1])

            def tail_out(ti):
                th0, tw, _ = TAILS[ti]
                nc.sync.dma_start(o_d[:, th0:th0 + tw, :], tails[ti]["osb"][:])

            # spine: exp -> ln(+1) per chunk; row sums + col sums trail
            for ci, (tch, h0, ch) in enumerate(tchs):
                sp = wp.tile([L, ch, L], fp32, tag=f"sp{ci}", name=f"sp{ci}")
                nc.scalar.activation(sp[:], tch[:], AF.Exp, bias=zb[:])
                if ci == 6:
                    tail_exp(0)     # expA slots into the ACT queue here
                nc.scalar.activation(sp[:], sp[:], AF.Ln, bias=ob[:])
                ti, off = tail_of(h0)
                assert tail_of(h0 + ch - 1)[0] == ti
                T = tails[ti]
                red = nc.vector if RED_ENG[ci] == "v" else nc.gpsimd
                red.tensor_reduce(
                    T["RS"][:, off:off + ch], sp[:],
                    axis=mybir.AxisListType.X, op=OP.add,
                )
                for j in range(ch):
                    nc.tensor.matmul(
                        T["cs"][:, off + j:off + j + 1],
                        sp[:, j, :],
                        ones[:, 0:1],
                        start=True, stop=True,
                    )
                for ti2, (th0, tw, after) in enumerate(TAILS[:2]):
                    if after == ci:
                        eng = nc.gpsimd if CHAIN_ENG[ti2] == "p" else nc.vector
                        tail_chain(ti2, eng)
                if ci == 4:
                    tail_chain_fin(0)   # DVE: bd_A - S1_A, clamp
                if ci == 5:
                    tail_final(0)
                    tail_out(0)

            tail_chain_fin(1)
            tail_chain(2, nc.gpsimd if CHAIN_ENG[2] == "p" else nc.vector)
            tail_chain_fin(2)
            tail_exp(1)
            tail_exp(2)
            tail_final(1)
            tail_out(1)
            tail_final(2)
            tail_out(2)

    nc.compile()
    return nc


def _softplus(x):
    return np.logaddexp(0.0, x)


def _core_inputs(s_edge, s_sib, c):
    b, hs = c >> 1, (c & 1) * H
    t = np.ascontiguousarray(s_sib[b, :, hs:hs + H, :], dtype=np.float32)
    d = np.arange(L)
    hl = np.arange(H)

    G = _softplus(t[d[:, None], hl[None, :], (hs + hl)[None, :]])
    DG = _softplus(t[d[:, None], hl[None, :], d[:, None]])
    ROWH = _softplus(
        s_sib[b, (hs + hl)[None, :], (hs + hl)[None, :], d[:, None]])

    E = (d[:, None] == (hs + hl)[None, :]).astype(np.float32)
    NF = 126.0 + E
    CN = LN2 * NF
    OME = 1.0 - E
    PD = (s_edge[b, :, hs:hs + H, 1]
          - s_edge[b, :, hs:hs + H, 0]).astype(np.float32)

    K1 = G + DG - E * G + CN
    K2 = ROWH + DG - E * DG + CN
    PDN = PD * (NF + 1.0)
    S0 = np.sum(PD * OME, axis=0, keepdims=True)   # [1, H] col sums
    Q2 = 2.0 * PD - E * PD - S0
    C6 = (Q2 + PD) * NF + (K1 - K2) * NF + K1 - 2.0 * K2 + PD

    hx = np.empty((L, C_COLS), dtype=np.float32)
    hx[:, C_C5:C_C5 + H] = PDN - K2
    hx[:, C_C7:C_C7 + H] = C6 / (NF + 2.0)
    hx[:, C_NF2:C_NF2 + H] = NF * NF
    hx[:, C_NFP1:C_NFP1 + H] = NF + 1.0
    hx[:, C_NFP2:C_NFP2 + H] = NF + 2.0
    hx[:, C_OME:C_OME + H] = OME
    return {"t": t, "hx": hx}


def make_in_maps(s_edge, s_sib):
    return [_core_inputs(s_edge, s_sib, c) for c in range(N_CORES)]


def get_program():
    global _PROGRAM
    if _PROGRAM is None:
        _PROGRAM = _build_program()
    return _PROGRAM


def assemble(results):
    out = np.empty((4, L, L, 2), dtype=np.float32)
    for c in range(N_CORES):
        b, hs = c >> 1, (c & 1) * H
        out[b, :, hs:hs + H, :] = results[c]["o"].reshape(L, H, 2)
    return out


def kernel(s_edge, s_sib, mask):
    from concourse.bass_utils import run_bass_kernel_spmd

    s_edge = np.asarray(s_edge)
    s_sib = np.asarray(s_sib)
    mask = np.asarray(mask)
    assert mask.all(), "kernel specialized for the spec's all-ones mask"

    nc = get_program()
    in_maps = make_in_maps(s_edge, s_sib)
    res = run_bass_kernel_spmd(nc, in_maps, list(range(N_CORES))).results
    return assemble(res)


# revision 9
# speedup vs baseline: 1.2624x; 1.0146x over previous
"""Trainium2 Bass kernel for LoopyBeliefPropagation (3-iter, mask=ones).

Math: for each (b, h) slice define tile[d,s] = s_sib[b,d,h,s],
SP = softplus(tile).  Collapsing the reference's 3-iteration loop into
closed form (see kernel_baseline.py) and folding every
stream-independent term into host constants leaves, per (d,h):

  RS[d,h] = sum_s SP[d,s]        (row-reduce of the stream, DVE/Pool)
  CS[d,h] = sum_s SP[s,d]        (col sums via ones matmuls, PE)
  r1 = CS + C5
  w  = r1 * OME,  S1 = colsum(w) broadcast   (PE matmul)
  bdiff = r1*NF2 + (CS + C7)*NFp2 - RS*NFp1 + w - S1
  out1 = sigmoid(bdiff) = 1/(1+q),  q = exp(-max(bdiff, -30))
  out0 = q * out1

C5, C7, NF2, NFp1, NFp2, OME are [L,H] host constants (they only touch
O(L*H) gathered values, not the 4 MiB stream).  softplus = Ln(Exp(x)+1)
using the natural_log_exp ACT table; the +1 rides the Ln bias.

Device schedule: the s_sib shard streams in ramped h-chunks; Exp/Ln run
back-to-back on ACT (the spine; a dummy Exp right after a memset pulls
the ACT table load to t~0).  The h-columns are independent, so the tail
algebra runs in three h-slices pipelined against the spine; work is
spread over DVE and the otherwise-idle GPSIMD (Pool) engine so only the
last narrow slice (plus its output DMA) trails the spine.

Sharding: 8 cores x (b in 0..3, h-half in {0:64, 64:128}).
"""

import numpy as np

L = 128
H = 64            # h-slices per core
N_CORES = 8
LN2 = float(np.log(2.0))

# ramped chunk sizes; tails cover h [0:32], [32:58], [58:64]
CHUNKS = [2, 6, 10, 14, 16, 10, 4, 2]
TAILS = [(0, 32, 3), (32, 26, 5), (58, 6, 7)]    # (h0, width, after-chunk)
RED_ENG = ["v", "v", "v", "v", "v", "v", "v", "v"]   # per-chunk row-reduce
CHAIN_ENG = ["p", "p", "v"]                          # per-tail algebra
assert sum(CHUNKS) == H

# host-constant column layout: 6 tensors of H columns each
C_C5 = 0 * H
C_C7 = 1 * H
C_NF2 = 2 * H
C_NFP1 = 3 * H
C_NFP2 = 4 * H
C_OME = 5 * H
C_COLS = 6 * H

_PROGRAM = None


def _build_program():
    import concourse.bacc as bacc
    import concourse.mybir as mybir
    import concourse.tile as tile

    fp32 = mybir.dt.float32
    AF = mybir.ActivationFunctionType
    OP = mybir.AluOpType

    # Exp and Ln live in one PWP table; without this filter the table
    # chooser maps Exp to exp_and_others and Ln to natural_log_exp_and_
    # others and reloads the ACT table (~1.3us) between every pair.
    if not getattr(bacc, "_lbp_act_tables_patched", False):
        _orig_tables = bacc.get_activation_tables

        def _ln_exp_only(arch):
            t = _orig_tables(arch)
            exp_ln = {AF.Exp, AF.Ln}
            return {
                name: (funcs if name == "natural_log_exp_and_others"
                       else set(funcs) - exp_ln)
                for name, funcs in t.items()
            }

        bacc.get_activation_tables = _ln_exp_only
        bacc._lbp_act_tables_patched = True

    nc = bacc.Bacc(None, target_bir_lowering=False)

    t_d = nc.dram_tensor("t", [L, H, L], fp32, kind="ExternalInput")
    hx_d = nc.dram_tensor("hx", [L, C_COLS], fp32, kind="ExternalInput")
    o_d = nc.dram_tensor("o", [L, H, 2], fp32, kind="ExternalOutput")

    with tile.TileContext(nc) as tc:
        with (
            tc.tile_pool(name="work", bufs=1) as wp,
            tc.tile_pool(name="psum", bufs=1, space="PSUM") as pp,
        ):
            zb = wp.tile([L, 1], fp32, tag="zb")
            ob = wp.tile([L, 1], fp32, tag="ob")
            ones = wp.tile([L, L], fp32, tag="ones")
            nc.gpsimd.memset(zb[:], 0.0)
            nc.gpsimd.memset(ob[:], 1.0)
            nc.gpsimd.memset(ones[:], 1.0)

            # dummy table-user: Bacc places the ACT table load right
            # before this, so it runs at t~0 instead of after chunk 0.
            dummy = wp.tile([L, 1], fp32, tag="dummy")
            nc.scalar.activation(dummy[:], zb[:], AF.Exp, bias=zb[:])

            # stream DMAs: chunk 0 first (critical-path opener); host
            # constants ride the queue behind chunk 3.
            hx = wp.tile([L, C_COLS], fp32, tag="hx")
            tchs = []
            h0 = 0
            for ci, ch in enumerate(CHUNKS):
                tch = wp.tile([L, ch, L], fp32, tag=f"tch{ci}")
                nc.sync.dma_start(tch[:], t_d[:, h0:h0 + ch, :])
                tchs.append((tch, h0, ch))
                if ci == 3:
                    nc.sync.dma_start(hx[:], hx_d[:])
                h0 += ch

            # per-tail state
            tails = []
            for ti, (th0, tw, _) in enumerate(TAILS):
                tails.append({
                    "RS": wp.tile([L, tw], fp32, tag=f"RS{ti}", name=f"RS{ti}"),
                    "cs": pp.tile([L, tw], fp32, tag=f"cs{ti}", name=f"cs{ti}"),
                    "bc": pp.tile([L, tw], fp32, tag=f"bc{ti}", name=f"bc{ti}"),
                    "osb": wp.tile([L, tw, 2], fp32, tag=f"osb{ti}",
                                   name=f"osb{ti}"),
                })

            def tail_of(h):
                for ti, (th0, tw, _) in enumerate(TAILS):
                    if th0 <= h < th0 + tw:
                        return ti, h - th0
                raise AssertionError

            def hxs(base, ti):
                th0, tw, _ = TAILS[ti]
                return hx[:, base + th0:base + th0 + tw]

            def tail_chain_pre(ti, eng, cs_src):
                """Algebra for tail slice ti up to bd-partial (pre -S1).

                cs_src: SBUF copy of CS for Pool (it cannot read PSUM);
                the PSUM-reading finish lives in tail_chain_fin.
                """
                th0, tw, _ = TAILS[ti]
                T = tails[ti]
                r1 = wp.tile([L, tw], fp32, tag=f"r1_{ti}", name=f"r1_{ti}")
                w_ = wp.tile([L, tw], fp32, tag=f"w_{ti}", name=f"w_{ti}")
                a = wp.tile([L, tw], fp32, tag=f"a_{ti}", name=f"a_{ti}")
                csc = wp.tile([L, tw], fp32, tag=f"csc_{ti}", name=f"csc_{ti}")
                b = wp.tile([L, tw], fp32, tag=f"b_{ti}", name=f"b_{ti}")
                rsn = wp.tile([L, tw], fp32, tag=f"rsn_{ti}", name=f"rsn_{ti}")
                bd = wp.tile([L, tw], fp32, tag=f"bd_{ti}", name=f"bd_{ti}")
                T["bd"] = bd
                T["bdc"] = wp.tile([L, tw], fp32, tag=f"bdc_{ti}",
                                   name=f"bdc_{ti}")
                T["q"] = wp.tile([L, tw], fp32, tag=f"q_{ti}", name=f"q_{ti}")
                T["s"] = wp.tile([L, tw], fp32, tag=f"s_{ti}", name=f"s_{ti}")
                eng.tensor_add(r1[:], cs_src[:], hxs(C_C5, ti))
                eng.tensor_mul(w_[:], r1[:], hxs(C_OME, ti))
                nc.tensor.matmul(T["bc"][:], ones[:], w_[:],
                                 start=True, stop=True)
                eng.tensor_mul(a[:], r1[:], hxs(C_NF2, ti))
                eng.tensor_add(csc[:], cs_src[:], hxs(C_C7, ti))
                eng.tensor_mul(b[:], csc[:], hxs(C_NFP2, ti))
                eng.tensor_mul(rsn[:], T["RS"][:], hxs(C_NFP1, ti))
                eng.tensor_add(bd[:], a[:], b[:])
                eng.tensor_sub(bd[:], bd[:], rsn[:])
                eng.tensor_add(bd[:], bd[:], w_[:])

            def tail_chain_fin(ti):
                """DVE finish: -S1 (PSUM read) and the exp-domain clamp."""
                T = tails[ti]
                nc.vector.tensor_sub(T["bdc"][:], T["bd"][:], T["bc"][:])
                nc.vector.tensor_scalar_max(T["bdc"][:], T["bdc"][:], -30.0)

            def tail_chain(ti, eng):
                if eng is nc.vector:
                    tail_chain_pre(ti, eng, tails[ti]["cs"])
                else:
                    th0, tw, _ = TAILS[ti]
                    cs_s = wp.tile([L, tw], fp32, tag=f"cs_s{ti}",
                                   name=f"cs_s{ti}")
                    nc.vector.tensor_copy(cs_s[:], tails[ti]["cs"][:])
                    tail_chain_pre(ti, eng, cs_s)

            def tail_exp(ti):
                T = tails[ti]
                nc.scalar.activation(T["q"][:], T["bdc"][:], AF.Exp,
                                     bias=zb[:], scale=-1.0)

            def tail_final(ti):
                T = tails[ti]
                nc.vector.tensor_scalar_add(T["s"][:], T["q"][:], 1.0)
                nc.vector.reciprocal(T["osb"][:, :, 1], T["s"][:])
                nc.vector.tensor_mul(T["osb"][:, :, 0], T["q"][:],
                                     T["osb"][:, :, 1])

            def tail_out(ti):
                th0, tw, _ = TAILS[ti]
                nc.sync.dma_start(o_d[:, th0:th0 + tw, :], tails[ti]["osb"][:])

            # spine: exp -> ln(+1) per chunk; row sums + col sums trail
            for ci, (tch, h0, ch) in enumerate(tchs):
                sp = wp.tile([L, ch, L], fp32, tag=f"sp{ci}", name=f"sp{ci}")
                nc.scalar.activation(sp[:], tch[:], AF.Exp, bias=zb[:])
                if ci == 6:
                    tail_exp(0)     # expA slots into the ACT queue here
                nc.scalar.activation(sp[:], sp[:], AF.Ln, bias=ob[:])
                ti, off = tail_of(h0)
                assert tail_of(h0 + ch - 1)[0] == ti
                T = tails[ti]
                red = nc.vector if RED_ENG[ci] == "v" else nc.gpsimd
                red.tensor_reduce(
                    T["RS"][:, off:off + ch], sp[:],
                    axis=mybir.AxisListType.X, op=OP.add,
                )
                for j in range(ch):
                    nc.tensor.matmul(
                        T["cs"][:, off + j:off + j + 1],
                        sp[:, j, :],
                        ones[:, 0:1],
                        start=True, stop=True,
                    )
                for ti2, (th0, tw, after) in enumerate(TAILS[:2]):
                    if after == ci:
                        eng = nc.gpsimd if CHAIN_ENG[ti2] == "p" else nc.vector
                        tail_chain(ti2, eng)
                if ci == 4:
                    tail_chain_fin(0)   # DVE: bd_A - S1_A, clamp
                if ci == 5:
                    tail_final(0)
                    tail_out(0)

            tail_chain(2, nc.gpsimd if CHAIN_ENG[2] == "p" else nc.vector)
            tail_chain_fin(2)
            tail_chain_fin(1)
            tail_exp(2)
            tail_exp(1)
            tail_final(2)
            tail_out(2)
            tail_final(1)
            tail_out(1)

    nc.compile()
    return nc


def _softplus(x):
    return np.logaddexp(0.0, x)


def _core_inputs(s_edge, s_sib, c):
    b, hs = c >> 1, (c & 1) * H
    t = np.ascontiguousarray(s_sib[b, :, hs:hs + H, :], dtype=np.float32)
    d = np.arange(L)
    hl = np.arange(H)

    G = _softplus(t[d[:, None], hl[None, :], (hs + hl)[None, :]])
    DG = _softplus(t[d[:, None], hl[None, :], d[:, None]])
    ROWH = _softplus(
        s_sib[b, (hs + hl)[None, :], (hs + hl)[None, :], d[:, None]])

    E = (d[:, None] == (hs + hl)[None, :]).astype(np.float32)
    NF = 126.0 + E
    CN = LN2 * NF
    OME = 1.0 - E
    PD = (s_edge[b, :, hs:hs + H, 1]
          - s_edge[b, :, hs:hs + H, 0]).astype(np.float32)

    K1 = G + DG - E * G + CN
    K2 = ROWH + DG - E * DG + CN
    PDN = PD * (NF + 1.0)
    S0 = np.sum(PD * OME, axis=0, keepdims=True)   # [1, H] col sums
    Q2 = 2.0 * PD - E * PD - S0
    C6 = (Q2 + PD) * NF + (K1 - K2) * NF + K1 - 2.0 * K2 + PD

    hx = np.empty((L, C_COLS), dtype=np.float32)
    hx[:, C_C5:C_C5 + H] = PDN - K2
    hx[:, C_C7:C_C7 + H] = C6 / (NF + 2.0)
    hx[:, C_NF2:C_NF2 + H] = NF * NF
    hx[:, C_NFP1:C_NFP1 + H] = NF + 1.0
    hx[:, C_NFP2:C_NFP2 + H] = NF + 2.0
    hx[:, C_OME:C_OME + H] = OME
    return {"t": t, "hx": hx}


def make_in_maps(s_edge, s_sib):
    return [_core_inputs(s_edge, s_sib, c) for c in range(N_CORES)]


def get_program():
    global _PROGRAM
    if _PROGRAM is None:
        _PROGRAM = _build_program()
    return _PROGRAM


def assemble(results):
    out = np.empty((4, L, L, 2), dtype=np.float32)
    for c in range(N_CORES):
        b, hs = c >> 1, (c & 1) * H
        out[b, :, hs:hs + H, :] = results[c]["o"].reshape(L, H, 2)
    return out


def kernel(s_edge, s_sib, mask):
    from concourse.bass_utils import run_bass_kernel_spmd

    s_edge = np.asarray(s_edge)
    s_sib = np.asarray(s_sib)
    mask = np.asarray(mask)
    assert mask.all(), "kernel specialized for the spec's all-ones mask"

    nc = get_program()
    in_maps = make_in_maps(s_edge, s_sib)
    res = run_bass_kernel_spmd(nc, in_maps, list(range(N_CORES))).results
    return assemble(res)


# revision 10
# speedup vs baseline: 1.2672x; 1.0038x over previous
"""Trainium2 Bass kernel for LoopyBeliefPropagation (3-iter, mask=ones).

Math: for each (b, h) slice define tile[d,s] = s_sib[b,d,h,s],
SP = softplus(tile).  Collapsing the reference's 3-iteration loop into
closed form (see kernel_baseline.py) and folding every
stream-independent term into host constants leaves, per (d,h):

  RS[d,h] = sum_s SP[d,s]        (row-reduce of the stream, DVE/Pool)
  CS[d,h] = sum_s SP[s,d]        (col sums via ones matmuls, PE)
  r1 = CS + C5
  w  = r1 * OME,  S1 = colsum(w) broadcast   (PE matmul)
  bdiff = r1*NF2 + (CS + C7)*NFp2 - RS*NFp1 + w - S1
  out1 = sigmoid(bdiff) = 1/(1+q),  q = exp(-max(bdiff, -30))
  out0 = q * out1

C5, C7, NF2, NFp1, NFp2, OME are [L,H] host constants (they only touch
O(L*H) gathered values, not the 4 MiB stream).  softplus = Ln(Exp(x)+1)
using the natural_log_exp ACT table; the +1 rides the Ln bias.

Device schedule: the s_sib shard streams in ramped h-chunks; Exp/Ln run
back-to-back on ACT (the spine; a dummy Exp right after a memset pulls
the ACT table load to t~0).  The h-columns are independent, so the tail
algebra runs in three h-slices pipelined against the spine; work is
spread over DVE and the otherwise-idle GPSIMD (Pool) engine so only the
last narrow slice (plus its output DMA) trails the spine.

Sharding: 8 cores x (b in 0..3, h-half in {0:64, 64:128}).
"""

import numpy as np

L = 128
H = 64            # h-slices per core
N_CORES = 8
LN2 = float(np.log(2.0))

# ramped chunk sizes; tails cover h [0:32], [32:58], [58:64]
CHUNKS = [2, 6, 10, 14, 16, 10, 4, 2]
TAILS = [(0, 32, 3), (32, 26, 5), (58, 6, 7)]    # (h0, width, after-chunk)
RED_ENG = ["v", "v", "v", "v", "v", "v", "v", "v"]   # per-chunk row-reduce
CHAIN_ENG = ["p", "p", "v"]                          # per-tail algebra
assert sum(CHUNKS) == H

# host-constant column layout: 6 tensors of H columns each
C_C5 = 0 * H
C_C7 = 1 * H
C_NF2 = 2 * H
C_NFP1 = 3 * H
C_NFP2 = 4 * H
C_OME = 5 * H
C_COLS = 6 * H

_PROGRAM = None


def _build_program():
    import concourse.bacc as bacc
    import concourse.mybir as mybir
    import concourse.tile as tile

    fp32 = mybir.dt.float32
    AF = mybir.ActivationFunctionType
    OP = mybir.AluOpType

    # Exp and Ln live in one PWP table; without this filter the table
    # chooser maps Exp to exp_and_others and Ln to natural_log_exp_and_
    # others and reloads the ACT table (~1.3us) between every pair.
    if not getattr(bacc, "_lbp_act_tables_patched", False):
        _orig_tables = bacc.get_activation_tables

        def _ln_exp_only(arch):
            t = _orig_tables(arch)
            exp_ln = {AF.Exp, AF.Ln}
            return {
                name: (funcs if name == "natural_log_exp_and_others"
                       else set(funcs) - exp_ln)
                for name, funcs in t.items()
            }

        bacc.get_activation_tables = _ln_exp_only
        bacc._lbp_act_tables_patched = True

    nc = bacc.Bacc(None, target_bir_lowering=False)

    t_d = nc.dram_tensor("t", [L, H, L], fp32, kind="ExternalInput")
    hx_d = nc.dram_tensor("hx", [L, C_COLS], fp32, kind="ExternalInput")
    o_d = nc.dram_tensor("o", [L, H, 2], fp32, kind="ExternalOutput")

    with tile.TileContext(nc) as tc:
        with (
            tc.tile_pool(name="work", bufs=1) as wp,
            tc.tile_pool(name="psum", bufs=1, space="PSUM") as pp,
        ):
            zb = wp.tile([L, 1], fp32, tag="zb")
            ob = wp.tile([L, 1], fp32, tag="ob")
            ones = wp.tile([L, L], fp32, tag="ones")
            nc.gpsimd.memset(zb[:], 0.0)
            nc.gpsimd.memset(ob[:], 1.0)
            nc.gpsimd.memset(ones[:], 1.0)

            # dummy table-user: Bacc places the ACT table load right
            # before this, so it runs at t~0 instead of after chunk 0.
            dummy = wp.tile([L, 1], fp32, tag="dummy")
            nc.scalar.activation(dummy[:], zb[:], AF.Exp, bias=zb[:])

            # stream DMAs: chunk 0 first (critical-path opener); host
            # constants ride the queue behind chunk 3.
            hx = wp.tile([L, C_COLS], fp32, tag="hx")
            tchs = []
            h0 = 0
            for ci, ch in enumerate(CHUNKS):
                tch = wp.tile([L, ch, L], fp32, tag=f"tch{ci}")
                nc.sync.dma_start(tch[:], t_d[:, h0:h0 + ch, :])
                tchs.append((tch, h0, ch))
                if ci == 3:
                    nc.sync.dma_start(hx[:], hx_d[:])
                h0 += ch

            # per-tail state
            tails = []
            for ti, (th0, tw, _) in enumerate(TAILS):
                tails.append({
                    "RS": wp.tile([L, tw], fp32, tag=f"RS{ti}", name=f"RS{ti}"),
                    "cs": pp.tile([L, tw], fp32, tag=f"cs{ti}", name=f"cs{ti}"),
                    "bc": pp.tile([L, tw], fp32, tag=f"bc{ti}", name=f"bc{ti}"),
                    "osb": wp.tile([L, tw, 2], fp32, tag=f"osb{ti}",
                                   name=f"osb{ti}"),
                })

            def tail_of(h):
                for ti, (th0, tw, _) in enumerate(TAILS):
                    if th0 <= h < th0 + tw:
                        return ti, h - th0
                raise AssertionError

            def hxs(base, ti):
                th0, tw, _ = TAILS[ti]
                return hx[:, base + th0:base + th0 + tw]

            def tail_chain_pre(ti, eng, cs_src):
                """Algebra for tail slice ti up to bd-partial (pre -S1).

                cs_src: SBUF copy of CS for Pool (it cannot read PSUM);
                the PSUM-reading finish lives in tail_chain_fin.
                """
                th0, tw, _ = TAILS[ti]
                T = tails[ti]
                r1 = wp.tile([L, tw], fp32, tag=f"r1_{ti}", name=f"r1_{ti}")
                w_ = wp.tile([L, tw], fp32, tag=f"w_{ti}", name=f"w_{ti}")
                a = wp.tile([L, tw], fp32, tag=f"a_{ti}", name=f"a_{ti}")
                csc = wp.tile([L, tw], fp32, tag=f"csc_{ti}", name=f"csc_{ti}")
                b = wp.tile([L, tw], fp32, tag=f"b_{ti}", name=f"b_{ti}")
                rsn = wp.tile([L, tw], fp32, tag=f"rsn_{ti}", name=f"rsn_{ti}")
                bd = wp.tile([L, tw], fp32, tag=f"bd_{ti}", name=f"bd_{ti}")
                T["bd"] = bd
                T["bdc"] = wp.tile([L, tw], fp32, tag=f"bdc_{ti}",
                                   name=f"bdc_{ti}")
                T["q"] = wp.tile([L, tw], fp32, tag=f"q_{ti}", name=f"q_{ti}")
                T["s"] = wp.tile([L, tw], fp32, tag=f"s_{ti}", name=f"s_{ti}")
                eng.tensor_add(r1[:], cs_src[:], hxs(C_C5, ti))
                eng.tensor_mul(w_[:], r1[:], hxs(C_OME, ti))
                nc.tensor.matmul(T["bc"][:], ones[:], w_[:],
                                 start=True, stop=True)
                eng.tensor_mul(a[:], r1[:], hxs(C_NF2, ti))
                eng.tensor_add(csc[:], cs_src[:], hxs(C_C7, ti))
                eng.tensor_mul(b[:], csc[:], hxs(C_NFP2, ti))
                eng.tensor_mul(rsn[:], T["RS"][:], hxs(C_NFP1, ti))
                eng.tensor_add(bd[:], a[:], b[:])
                eng.tensor_sub(bd[:], bd[:], rsn[:])
                eng.tensor_add(bd[:], bd[:], w_[:])

            def tail_chain_fin(ti):
                """DVE finish: -S1 (PSUM read) and the exp-domain clamp."""
                T = tails[ti]
                nc.vector.tensor_sub(T["bdc"][:], T["bd"][:], T["bc"][:])
                nc.vector.tensor_scalar_max(T["bdc"][:], T["bdc"][:], -30.0)

            def tail_chain(ti, eng):
                if eng is nc.vector:
                    tail_chain_pre(ti, eng, tails[ti]["cs"])
                else:
                    th0, tw, _ = TAILS[ti]
                    cs_s = wp.tile([L, tw], fp32, tag=f"cs_s{ti}",
                                   name=f"cs_s{ti}")
                    nc.vector.tensor_copy(cs_s[:], tails[ti]["cs"][:])
                    tail_chain_pre(ti, eng, cs_s)

            def tail_exp(ti):
                T = tails[ti]
                nc.scalar.activation(T["q"][:], T["bdc"][:], AF.Exp,
                                     bias=zb[:], scale=-1.0)

            def tail_final(ti):
                T = tails[ti]
                nc.vector.tensor_scalar_add(T["s"][:], T["q"][:], 1.0)
                nc.vector.reciprocal(T["osb"][:, :, 1], T["s"][:])
                nc.vector.tensor_mul(T["osb"][:, :, 0], T["q"][:],
                                     T["osb"][:, :, 1])

            def tail_out(ti):
                th0, tw, _ = TAILS[ti]
                nc.sync.dma_start(o_d[:, th0:th0 + tw, :], tails[ti]["osb"][:])

            # spine: exp -> ln(+1) per chunk; row sums + col sums trail
            for ci, (tch, h0, ch) in enumerate(tchs):
                sp = wp.tile([L, ch, L], fp32, tag=f"sp{ci}", name=f"sp{ci}")
                if ci == 5:
                    with tc.tile_wait_until(0.0153):
                        nc.scalar.activation(sp[:], tch[:], AF.Exp, bias=zb[:])
                else:
                    nc.scalar.activation(sp[:], tch[:], AF.Exp, bias=zb[:])
                if ci == 6:
                    tail_exp(0)     # expA slots into the ACT queue here
                nc.scalar.activation(sp[:], sp[:], AF.Ln, bias=ob[:])
                ti, off = tail_of(h0)
                assert tail_of(h0 + ch - 1)[0] == ti
                T = tails[ti]
                red = nc.vector if RED_ENG[ci] == "v" else nc.gpsimd
                red.tensor_reduce(
                    T["RS"][:, off:off + ch], sp[:],
                    axis=mybir.AxisListType.X, op=OP.add,
                )
                for j in range(ch):
                    nc.tensor.matmul(
                        T["cs"][:, off + j:off + j + 1],
                        sp[:, j, :],
                        ones[:, 0:1],
                        start=True, stop=True,
                    )
                for ti2, (th0, tw, after) in enumerate(TAILS[:2]):
                    if after == ci:
                        eng = nc.gpsimd if CHAIN_ENG[ti2] == "p" else nc.vector
                        tail_chain(ti2, eng)
                if ci == 4:
                    tail_chain_fin(0)   # DVE: bd_A - S1_A, clamp
                if ci == 5:
                    tail_final(0)
                    tail_out(0)

            tail_chain(2, nc.gpsimd if CHAIN_ENG[2] == "p" else nc.vector)
            tail_chain_fin(2)
            tail_chain_fin(1)
            tail_exp(2)
            tail_exp(1)
            tail_final(2)
            tail_out(2)
            tail_final(1)
            tail_out(1)

    nc.compile()
    return nc


def _softplus(x):
    return np.logaddexp(0.0, x)


def _core_inputs(s_edge, s_sib, c):
    b, hs = c >> 1, (c & 1) * H
    t = np.ascontiguousarray(s_sib[b, :, hs:hs + H, :], dtype=np.float32)
    d = np.arange(L)
    hl = np.arange(H)

    G = _softplus(t[d[:, None], hl[None, :], (hs + hl)[None, :]])
    DG = _softplus(t[d[:, None], hl[None, :], d[:, None]])
    ROWH = _softplus(
        s_sib[b, (hs + hl)[None, :], (hs + hl)[None, :], d[:, None]])

    E = (d[:, None] == (hs + hl)[None, :]).astype(np.float32)
    NF = 126.0 + E
    CN = LN2 * NF
    OME = 1.0 - E
    PD = (s_edge[b, :, hs:hs + H, 1]
          - s_edge[b, :, hs:hs + H, 0]).astype(np.float32)

    K1 = G + DG - E * G + CN
    K2 = ROWH + DG - E * DG + CN
    PDN = PD * (NF + 1.0)
    S0 = np.sum(PD * OME, axis=0, keepdims=True)   # [1, H] col sums
    Q2 = 2.0 * PD - E * PD - S0
    C6 = (Q2 + PD) * NF + (K1 - K2) * NF + K1 - 2.0 * K2 + PD

    hx = np.empty((L, C_COLS), dtype=np.float32)
    hx[:, C_C5:C_C5 + H] = PDN - K2
    hx[:, C_C7:C_C7 + H] = C6 / (NF + 2.0)
    hx[:, C_NF2:C_NF2 + H] = NF * NF
    hx[:, C_NFP1:C_NFP1 + H] = NF + 1.0
    hx[:, C_NFP2:C_NFP2 + H] = NF + 2.0
    hx[:, C_OME:C_OME + H] = OME
    return {"t": t, "hx": hx}


def make_in_maps(s_edge, s_sib):
    return [_core_inputs(s_edge, s_sib, c) for c in range(N_CORES)]


def get_program():
    global _PROGRAM
    if _PROGRAM is None:
        _PROGRAM = _build_program()
    return _PROGRAM


def assemble(results):
    out = np.empty((4, L, L, 2), dtype=np.float32)
    for c in range(N_CORES):
        b, hs = c >> 1, (c & 1) * H
        out[b, :, hs:hs + H, :] = results[c]["o"].reshape(L, H, 2)
    return out


def kernel(s_edge, s_sib, mask):
    from concourse.bass_utils import run_bass_kernel_spmd

    s_edge = np.asarray(s_edge)
    s_sib = np.asarray(s_sib)
    mask = np.asarray(mask)
    assert mask.all(), "kernel specialized for the spec's all-ones mask"

    nc = get_program()
    in_maps = make_in_maps(s_edge, s_sib)
    res = run_bass_kernel_spmd(nc, in_maps, list(range(N_CORES))).results
    return assemble(res)


# revision 11
# speedup vs baseline: 1.2847x; 1.0138x over previous
"""Trainium2 Bass kernel for LoopyBeliefPropagation (3-iter, mask=ones).

Math: for each (b, h) slice define tile[d,s] = s_sib[b,d,h,s],
SP = softplus(tile).  Collapsing the reference's 3-iteration loop into
closed form (see kernel_baseline.py) and folding every
stream-independent term into host constants leaves, per (d,h):

  RS[d,h] = sum_s SP[d,s]        (row-reduce of the stream, DVE/Pool)
  CS[d,h] = sum_s SP[s,d]        (col sums via ones matmuls, PE)
  r1 = CS + C5
  w  = r1 * OME,  S1 = colsum(w) broadcast   (PE matmul)
  bdiff = r1*NF2 + (CS + C7)*NFp2 - RS*NFp1 + w - S1
  out1 = sigmoid(bdiff) = 1/(1+q),  q = exp(-max(bdiff, -30))
  out0 = q * out1

C5, C7, NF2, NFp1, NFp2, OME are [L,H] host constants (they only touch
O(L*H) gathered values, not the 4 MiB stream).  softplus = Ln(Exp(x)+1)
using the natural_log_exp ACT table; the +1 rides the Ln bias.

Device schedule: the s_sib shard streams in ramped h-chunks; Exp/Ln run
back-to-back on ACT (the spine; a dummy Exp right after a memset pulls
the ACT table load to t~0).  The h-columns are independent, so the tail
algebra runs in three h-slices pipelined against the spine; work is
spread over DVE and the otherwise-idle GPSIMD (Pool) engine so only the
last narrow slice (plus its output DMA) trails the spine.

Sharding: 8 cores x (b in 0..3, h-half in {0:64, 64:128}).
"""

import numpy as np

L = 128
H = 64            # h-slices per core
N_CORES = 8
LN2 = float(np.log(2.0))

# ramped chunk sizes; tails cover h [0:32], [32:58], [58:64]
CHUNKS = [2, 6, 10, 14, 16, 10, 4, 2]
TAILS = [(0, 32, 3), (32, 26, 5), (58, 6, 7)]    # (h0, width, after-chunk)
RED_ENG = ["v", "v", "v", "v", "v", "v", "v", "v"]   # per-chunk row-reduce
CHAIN_ENG = ["p", "p", "v"]                          # per-tail algebra
assert sum(CHUNKS) == H

# host-constant column layout: 6 tensors of H columns each
C_C5 = 0 * H
C_C7 = 1 * H
C_NF2 = 2 * H
C_NFP1 = 3 * H
C_NFP2 = 4 * H
C_OME = 5 * H
C_COLS = 6 * H

_PROGRAM = None


def _build_program():
    import concourse.bacc as bacc
    import concourse.mybir as mybir
    import concourse.tile as tile

    fp32 = mybir.dt.float32
    AF = mybir.ActivationFunctionType
    OP = mybir.AluOpType

    # Exp and Ln live in one PWP table; without this filter the table
    # chooser maps Exp to exp_and_others and Ln to natural_log_exp_and_
    # others and reloads the ACT table (~1.3us) between every pair.
    if not getattr(bacc, "_lbp_act_tables_patched", False):
        _orig_tables = bacc.get_activation_tables

        def _ln_exp_only(arch):
            t = _orig_tables(arch)
            exp_ln = {AF.Exp, AF.Ln}
            return {
                name: (funcs if name == "natural_log_exp_and_others"
                       else set(funcs) - exp_ln)
                for name, funcs in t.items()
            }

        bacc.get_activation_tables = _ln_exp_only
        bacc._lbp_act_tables_patched = True

    nc = bacc.Bacc(None, target_bir_lowering=False)

    t_d = nc.dram_tensor("t", [L, H, L], fp32, kind="ExternalInput")
    hx_d = nc.dram_tensor("hx", [L, C_COLS], fp32, kind="ExternalInput")
    o_d = nc.dram_tensor("o", [L, H, 2], fp32, kind="ExternalOutput")

    with tile.TileContext(nc) as tc:
        with (
            tc.tile_pool(name="work", bufs=1) as wp,
            tc.tile_pool(name="psum", bufs=1, space="PSUM") as pp,
        ):
            zb = wp.tile([L, 1], fp32, tag="zb")
            ob = wp.tile([L, 1], fp32, tag="ob")
            ones = wp.tile([L, L], fp32, tag="ones")
            nc.gpsimd.memset(zb[:], 0.0)
            nc.gpsimd.memset(ob[:], 1.0)
            nc.gpsimd.memset(ones[:], 1.0)

            # dummy table-user: Bacc places the ACT table load right
            # before this, so it runs at t~0 instead of after chunk 0.
            dummy = wp.tile([L, 1], fp32, tag="dummy")
            nc.scalar.activation(dummy[:], zb[:], AF.Exp, bias=zb[:])

            # stream DMAs: chunk 0 first (critical-path opener); host
            # constants ride the queue behind chunk 3.
            hx = wp.tile([L, C_COLS], fp32, tag="hx")
            tchs = []
            h0 = 0
            for ci, ch in enumerate(CHUNKS):
                tch = wp.tile([L, ch, L], fp32, tag=f"tch{ci}")
                nc.sync.dma_start(tch[:], t_d[:, h0:h0 + ch, :])
                tchs.append((tch, h0, ch))
                if ci == 3:
                    nc.sync.dma_start(hx[:], hx_d[:])
                h0 += ch

            # per-tail state
            tails = []
            for ti, (th0, tw, _) in enumerate(TAILS):
                tails.append({
                    "RS": wp.tile([L, tw], fp32, tag=f"RS{ti}", name=f"RS{ti}"),
                    "cs": pp.tile([L, tw], fp32, tag=f"cs{ti}", name=f"cs{ti}"),
                    "bc": pp.tile([L, tw], fp32, tag=f"bc{ti}", name=f"bc{ti}"),
                    "osb": wp.tile([L, tw, 2], fp32, tag=f"osb{ti}",
                                   name=f"osb{ti}"),
                })

            def tail_of(h):
                for ti, (th0, tw, _) in enumerate(TAILS):
                    if th0 <= h < th0 + tw:
                        return ti, h - th0
                raise AssertionError

            def hxs(base, ti):
                th0, tw, _ = TAILS[ti]
                return hx[:, base + th0:base + th0 + tw]

            def tail_chain_pre(ti, eng, cs_src):
                """Algebra for tail slice ti up to bd-partial (pre -S1).

                cs_src: SBUF copy of CS for Pool (it cannot read PSUM);
                the PSUM-reading finish lives in tail_chain_fin.
                """
                th0, tw, _ = TAILS[ti]
                T = tails[ti]
                r1 = wp.tile([L, tw], fp32, tag=f"r1_{ti}", name=f"r1_{ti}")
                w_ = wp.tile([L, tw], fp32, tag=f"w_{ti}", name=f"w_{ti}")
                a = wp.tile([L, tw], fp32, tag=f"a_{ti}", name=f"a_{ti}")
                csc = wp.tile([L, tw], fp32, tag=f"csc_{ti}", name=f"csc_{ti}")
                b = wp.tile([L, tw], fp32, tag=f"b_{ti}", name=f"b_{ti}")
                rsn = wp.tile([L, tw], fp32, tag=f"rsn_{ti}", name=f"rsn_{ti}")
                bd = wp.tile([L, tw], fp32, tag=f"bd_{ti}", name=f"bd_{ti}")
                T["bd"] = bd
                T["bdc"] = wp.tile([L, tw], fp32, tag=f"bdc_{ti}",
                                   name=f"bdc_{ti}")
                T["q"] = wp.tile([L, tw], fp32, tag=f"q_{ti}", name=f"q_{ti}")
                T["s"] = wp.tile([L, tw], fp32, tag=f"s_{ti}", name=f"s_{ti}")
                eng.tensor_add(r1[:], cs_src[:], hxs(C_C5, ti))
                eng.tensor_mul(w_[:], r1[:], hxs(C_OME, ti))
                nc.tensor.matmul(T["bc"][:], ones[:], w_[:],
                                 start=True, stop=True)
                eng.tensor_mul(a[:], r1[:], hxs(C_NF2, ti))
                eng.tensor_add(csc[:], cs_src[:], hxs(C_C7, ti))
                eng.tensor_mul(b[:], csc[:], hxs(C_NFP2, ti))
                eng.tensor_mul(rsn[:], T["RS"][:], hxs(C_NFP1, ti))
                eng.tensor_add(bd[:], a[:], b[:])
                eng.tensor_sub(bd[:], bd[:], rsn[:])
                eng.tensor_add(bd[:], bd[:], w_[:])

            def tail_chain_fin(ti):
                """DVE finish: -S1 (PSUM read) and the exp-domain clamp."""
                T = tails[ti]
                nc.vector.tensor_sub(T["bdc"][:], T["bd"][:], T["bc"][:])
                nc.vector.tensor_scalar_max(T["bdc"][:], T["bdc"][:], -30.0)

            def tail_chain(ti, eng):
                if eng is nc.vector:
                    tail_chain_pre(ti, eng, tails[ti]["cs"])
                else:
                    th0, tw, _ = TAILS[ti]
                    cs_s = wp.tile([L, tw], fp32, tag=f"cs_s{ti}",
                                   name=f"cs_s{ti}")
                    nc.vector.tensor_copy(cs_s[:], tails[ti]["cs"][:])
                    tail_chain_pre(ti, eng, cs_s)

            def tail_exp(ti):
                T = tails[ti]
                nc.scalar.activation(T["q"][:], T["bdc"][:], AF.Exp,
                                     bias=zb[:], scale=-1.0)

            def tail_final(ti):
                T = tails[ti]
                nc.vector.tensor_scalar_add(T["s"][:], T["q"][:], 1.0)
                nc.vector.reciprocal(T["osb"][:, :, 1], T["s"][:])
                nc.vector.tensor_mul(T["osb"][:, :, 0], T["q"][:],
                                     T["osb"][:, :, 1])

            def tail_out(ti):
                th0, tw, _ = TAILS[ti]
                nc.sync.dma_start(o_d[:, th0:th0 + tw, :], tails[ti]["osb"][:])

            # spine: exp -> ln(+1) per chunk; row sums + col sums trail
            for ci, (tch, h0, ch) in enumerate(tchs):
                sp = wp.tile([L, ch, L], fp32, tag=f"sp{ci}", name=f"sp{ci}")
                if ci in (5, 6):
                    with tc.tile_wait_until({5: 0.0153, 6: 0.0184}[ci]):
                        nc.scalar.activation(sp[:], tch[:], AF.Exp, bias=zb[:])
                else:
                    nc.scalar.activation(sp[:], tch[:], AF.Exp, bias=zb[:])
                if ci == 6:
                    tail_exp(0)     # expA slots into the ACT queue here
                nc.scalar.activation(sp[:], sp[:], AF.Ln, bias=ob[:])
                ti, off = tail_of(h0)
                assert tail_of(h0 + ch - 1)[0] == ti
                T = tails[ti]
                red = nc.vector if RED_ENG[ci] == "v" else nc.gpsimd
                red.tensor_reduce(
                    T["RS"][:, off:off + ch], sp[:],
                    axis=mybir.AxisListType.X, op=OP.add,
                )
                for j in range(ch):
                    nc.tensor.matmul(
                        T["cs"][:, off + j:off + j + 1],
                        sp[:, j, :],
                        ones[:, 0:1],
                        start=True, stop=True,
                    )
                for ti2, (th0, tw, after) in enumerate(TAILS[:2]):
                    if after == ci:
                        eng = nc.gpsimd if CHAIN_ENG[ti2] == "p" else nc.vector
                        tail_chain(ti2, eng)
                if ci == 4:
                    tail_chain_fin(0)   # DVE: bd_A - S1_A, clamp
                if ci == 5:
                    tail_final(0)
                    tail_out(0)

            tail_chain(2, nc.gpsimd if CHAIN_ENG[2] == "p" else nc.vector)
            tail_chain_fin(2)
            tail_chain_fin(1)
            tail_exp(2)
            tail_exp(1)
            tail_final(2)
            tail_out(2)
            tail_final(1)
            tail_out(1)

    nc.compile()
    return nc


def _softplus(x):
    return np.logaddexp(0.0, x)


def _core_inputs(s_edge, s_sib, c):
    b, hs = c >> 1, (c & 1) * H
    t = np.ascontiguousarray(s_sib[b, :, hs:hs + H, :], dtype=np.float32)
    d = np.arange(L)
    hl = np.arange(H)

    G = _softplus(t[d[:, None], hl[None, :], (hs + hl)[None, :]])
    DG = _softplus(t[d[:, None], hl[None, :], d[:, None]])
    ROWH = _softplus(
        s_sib[b, (hs + hl)[None, :], (hs + hl)[None, :], d[:, None]])

    E = (d[:, None] == (hs + hl)[None, :]).astype(np.float32)
    NF = 126.0 + E
    CN = LN2 * NF
    OME = 1.0 - E
    PD = (s_edge[b, :, hs:hs + H, 1]
          - s_edge[b, :, hs:hs + H, 0]).astype(np.float32)

    K1 = G + DG - E * G + CN
    K2 = ROWH + DG - E * DG + CN
    PDN = PD * (NF + 1.0)
    S0 = np.sum(PD * OME, axis=0, keepdims=True)   # [1, H] col sums
    Q2 = 2.0 * PD - E * PD - S0
    C6 = (Q2 + PD) * NF + (K1 - K2) * NF + K1 - 2.0 * K2 + PD

    hx = np.empty((L, C_COLS), dtype=np.float32)
    hx[:, C_C5:C_C5 + H] = PDN - K2
    hx[:, C_C7:C_C7 + H] = C6 / (NF + 2.0)
    hx[:, C_NF2:C_NF2 + H] = NF * NF
    hx[:, C_NFP1:C_NFP1 + H] = NF + 1.0
    hx[:, C_NFP2:C_NFP2 + H] = NF + 2.0
    hx[:, C_OME:C_OME + H] = OME
    return {"t": t, "hx": hx}


def make_in_maps(s_edge, s_sib):
    return [_core_inputs(s_edge, s_sib, c) for c in range(N_CORES)]


def get_program():
    global _PROGRAM
    if _PROGRAM is None:
        _PROGRAM = _build_program()
    return _PROGRAM


def assemble(results):
    out = np.empty((4, L, L, 2), dtype=np.float32)
    for c in range(N_CORES):
        b, hs = c >> 1, (c & 1) * H
        out[b, :, hs:hs + H, :] = results[c]["o"].reshape(L, H, 2)
    return out


def kernel(s_edge, s_sib, mask):
    from concourse.bass_utils import run_bass_kernel_spmd

    s_edge = np.asarray(s_edge)
    s_sib = np.asarray(s_sib)
    mask = np.asarray(mask)
    assert mask.all(), "kernel specialized for the spec's all-ones mask"

    nc = get_program()
    in_maps = make_in_maps(s_edge, s_sib)
    res = run_bass_kernel_spmd(nc, in_maps, list(range(N_CORES))).results
    return assemble(res)
